# revision 16
# baseline (speedup 1.0000x reference)
"""Trainium2 Bass kernel for nn_DecoderBlock (S=4096, D=768, H=12).

Strategy (8 NeuronCores, SPMD, sequence-parallel: core c owns rows
[c*512, (c+1)*512)):

  - All activations transposed (features on partitions, sequence free).
    LayerNorm stats via ones-vector matmuls (partition reduction on PE),
    computed on bf16 copies (1 cyc/row instead of fp32's 4).
  - Attention runs in fp8(e4m3) with DoubleRow matmuls (0.5 cyc/row):
      * scores: per head, the 64-dim contraction is split into a 2x32
        DoubleRow pair; k/q are produced in a "paired" layout
        [32*(h%4)+d32, i, s] (4 heads per 128 partitions) via host-side
        weight-column permutation.
      * PV: DoubleRow pairs two key-tiles per matmul; the stationary V
        carries a 65th all-ones column so the softmax denominator
        accumulates on PSUM partition 64 for free (no M=1 matmuls).
  - Softmax exp splits across TWO engines: the Scalar engine computes
    lam*exp(y) (bias=ln lam) for even key-tiles while the Vector engine
    computes the same for odd tiles via a custom 6-stage DVE op
    (((c0*y+c1)^2+c2)^2)^2 ~= lam*exp(y) (+-2.8% shape error; the
    common lam cancels in the softmax normalization).
  - x and qkv weights are pre-scaled x8 (fp8 denormal avoidance), so
    q8/k8/v8 = 64*(true); scores psum = 4096*s_true; the 1/64 folds
    into the softmax-denominator reciprocal, the 1/4096 and 1/sqrt(64)
    into the exp input scale.
  - K (paired) and V (natural) are cast to fp8 and exchanged with 3
    chunked AllGathers (one per 4-head group) so attention on group g
    starts while later chunks are in flight.
  - FFN stays bf16 (fp8 would blow the 2e-2 error budget); its weights
    are prefetched during attention.
"""

import os
import sys

for _p in ("/opt/trn_rl_repo", os.path.expanduser("~/.axon_site/_ro/trn_rl_repo")):
    if os.path.isdir(_p) and _p not in sys.path:
        sys.path.append(_p)

import numpy as np
from contextlib import ExitStack

import concourse.bass as bass
import concourse.tile as tile
from concourse import bacc, mybir
import concourse.dve_ops as dve_ops
from concourse.dve_spec import Spec, Src0, C0, C1, C2, sq, lower as dve_lower
from concourse.dve_uop import DveOpSpec

F32 = mybir.dt.float32
BF16 = mybir.dt.bfloat16
FP8 = mybir.dt.float8e4
AF = mybir.ActivationFunctionType
ALU = mybir.AluOpType
DR = mybir.MatmulPerfMode.DoubleRow

# exp-poly coefficients: ((EC0*y+EC1)^2+EC2)^4 ~= ELAM * e^y on |y|<=2.2
EC0, EC1, EC2 = 0.19508098, 0.82453421, 0.56463811
ELAM = 2.3759038641914842


def _ref_exp_poly(in0, in1, s0, s1, imm2):
    t = (in0.astype(np.float32) * s0 + s1) ** 2 + imm2
    return (t * t) ** 2


def register_exp_poly():
    for op in dve_ops.OPS:
        if op.name == "EXP_POLY_ANT":
            return op
    spec = Spec(body=sq(sq(sq(Src0 * C0 + C1) + C2)), reference=_ref_exp_poly)
    name = "EXP_POLY_ANT"
    opcode = dve_ops._CUSTOM_DVE_ROW_BASE + len(dve_ops.OPS)
    shas = {}
    for ver in ("v3", "v4"):
        try:
            s = DveOpSpec(name=name, opcode=opcode,
                          uops=dve_lower(spec, ver=ver), rd1_en=False)
            shas[ver] = s.sha(ver)
        except Exception:
            pass
    op = dve_ops.DveOp(name, spec, subdim=False, uops_sha=shas)
    dve_ops.OPS.append(op)
    dve_ops.CUSTOM_DVE_SPECS[name] = op.spec
    dve_ops._SUB_OPCODE_FOR_NAME[name] = opcode
    return op


EXP_POLY = register_exp_poly()


class Cfg:
    def __init__(self, S=4096, D=768, H=12, NC=8, eps=1e-5):
        self.S, self.D, self.H, self.NC, self.eps = S, D, H, NC, eps
        self.DH = D // H              # 64
        self.SL = S // NC             # 512 local rows
        self.ND = D // 128            # 6 d-tiles
        self.NDP = self.ND // 2       # 3 d-tile pairs
        self.NSK = S // 128           # 32 key tiles
        self.NPAIR = self.NSK // 2    # 16 key-tile pairs
        self.NG = 3                   # head groups (4 heads each)
        self.NFF = 4 * D // 128       # 24 ffn tiles
        self.scale = 1.0 / float(np.sqrt(self.DH))
        self.yscale = self.scale / 1024.0   # q8,k8 are 32x the true values


def build(cfg: Cfg, debug=False, enable_asserts=False, dbg_dump=False):
    nc = bacc.Bacc(
        "TRN2",
        target_bir_lowering=False,
        debug=debug,
        enable_asserts=enable_asserts,
        num_devices=cfg.NC,
    )
    S, D, H, SL = cfg.S, cfg.D, cfg.H, cfg.SL
    ND, NDP, NSK, NPAIR, NG, NFF, NC = (cfg.ND, cfg.NDP, cfg.NSK, cfg.NPAIR,
                                        cfg.NG, cfg.NFF, cfg.NC)

    # ---- DRAM I/O ----------------------------------------------------------
    xT = nc.dram_tensor("xT", [D, SL], F32, kind="ExternalInput").ap()
    w_qkv8 = nc.dram_tensor("w_qkv8", [NDP * 128, 2 * 2304], FP8,
                            kind="ExternalInput").ap()
    b_q64 = nc.dram_tensor("b_q64", [128, ND], F32, kind="ExternalInput").ap()
    b_k64 = nc.dram_tensor("b_k64", [128, ND], F32, kind="ExternalInput").ap()
    b_v = nc.dram_tensor("b_v", [1, D], F32, kind="ExternalInput").ap()
    ln1w = nc.dram_tensor("ln1w", [128, ND], F32, kind="ExternalInput").ap()
    ln1b = nc.dram_tensor("ln1b", [128, ND], F32, kind="ExternalInput").ap()
    ln2w = nc.dram_tensor("ln2w", [128, ND], F32, kind="ExternalInput").ap()
    ln2b = nc.dram_tensor("ln2b", [128, ND], F32, kind="ExternalInput").ap()
    w_fcT = nc.dram_tensor("w_fcT", [D, 4 * D], BF16, kind="ExternalInput").ap()
    b_fc = nc.dram_tensor("b_fc", [128, NFF], F32, kind="ExternalInput").ap()
    w_projT = nc.dram_tensor("w_projT", [4 * D, D], BF16,
                             kind="ExternalInput").ap()
    b_proj = nc.dram_tensor("b_proj", [128, ND], F32, kind="ExternalInput").ap()
    outT = nc.dram_tensor("outT", [D, SL], F32, kind="ExternalOutput").ap()
    if dbg_dump:
        d_ln1x = nc.dram_tensor("d_ln1x", [128, SL], F32, kind="ExternalOutput").ap()
        d_ln1x8 = nc.dram_tensor("d_ln1x8", [128, 6 * SL], FP8, kind="ExternalOutput").ap()
        d_q8 = nc.dram_tensor("d_q8", [128, 2 * SL], FP8, kind="ExternalOutput").ap()
        d_kT = nc.dram_tensor("d_kT", [128, 2 * S], FP8, kind="ExternalOutput").ap()
        d_vall = nc.dram_tensor("d_vall", [128, H * NSK * 64], FP8, kind="ExternalOutput").ap()
        d_ex = nc.dram_tensor("d_ex", [128, 2 * SL], FP8, kind="ExternalOutput").ap()
        d_ctxS = nc.dram_tensor("d_ctxS", [64, SL], F32, kind="ExternalOutput").ap()
        d_denr = nc.dram_tensor("d_denr", [1, SL], F32, kind="ExternalOutput").ap()
        d_x2 = nc.dram_tensor("d_x2", [128, SL], F32, kind="ExternalOutput").ap()

    with tile.TileContext(nc) as tc, ExitStack() as top:
        persist = top.enter_context(tc.tile_pool(name="persist", bufs=1))
        dram = top.enter_context(tc.tile_pool(name="dram", bufs=1, space="DRAM"))

        ones_colb = persist.tile([128, 1], BF16)
        nc.vector.memset(ones_colb[:], 1.0)
        ones_row32 = persist.tile([1, 128], F32)
        nc.vector.memset(ones_row32[:], 32.0)
        ones128b = persist.tile([1, 128], BF16)
        nc.vector.memset(ones128b[:], 1.0)
        eps_tile = persist.tile([1, 1], F32)
        nc.vector.memset(eps_tile[:], float(cfg.eps))
        lnlam = persist.tile([128, 1], F32)
        nc.vector.memset(lnlam[:], float(np.log(ELAM)))

        b_q64_sb = persist.tile([128, ND], F32)
        nc.sync.dma_start(b_q64_sb[:], b_q64[:])
        b_k64_sb = persist.tile([128, ND], F32)
        nc.sync.dma_start(b_k64_sb[:], b_k64[:])
        b_v_sb = persist.tile([1, D], F32)
        nc.sync.dma_start(b_v_sb[:], b_v[:])
        ln1w_sb = persist.tile([128, ND], F32)
        nc.sync.dma_start(ln1w_sb[:], ln1w[:])
        ln1b_sb = persist.tile([128, ND], F32)
        nc.sync.dma_start(ln1b_sb[:], ln1b[:])
        ln2w_sb = persist.tile([128, ND], F32)
        nc.sync.dma_start(ln2w_sb[:], ln2w[:])
        ln2b_sb = persist.tile([128, ND], F32)
        nc.sync.dma_start(ln2b_sb[:], ln2b[:])
        b_fc_sb = persist.tile([128, NFF], F32)
        nc.sync.dma_start(b_fc_sb[:], b_fc[:])
        b_proj_sb = persist.tile([128, ND], F32)
        nc.sync.dma_start(b_proj_sb[:], b_proj[:])

        ln1w8 = persist.tile([128, ND], F32)
        nc.vector.tensor_scalar_mul(ln1w8[:], ln1w_sb[:], 8.0)
        ln1b8 = persist.tile([128, ND], F32)
        nc.vector.tensor_scalar_mul(ln1b8[:], ln1b_sb[:], 8.0)

        # persistent activations
        ln1x = [persist.tile([128, SL], F32, name=f"ln1x{t}") for t in range(ND)]
        x2 = [persist.tile([128, SL], F32, name=f"x2_{t}") for t in range(ND)]
        q8 = [persist.tile([128, 2 * SL], FP8, name=f"q8_{g}") for g in range(NG)]

        def layernorm_T(src_f32, src_bf, w_sb, b_sb, wq_sb, bq_sb, q_dtype,
                        out_f32, out_q):
            """LN over the partition (feature) axis.

            src_bf: bf16 copies of src_f32 (for 1cyc/row stat matmuls).
            out_f32[t] = LN(x)*w+b (f32); out_q block tile gets
            LN(x)*wq+bq in q_dtype (cols t*SL..(t+1)*SL)."""
            with tc.tile_pool(name="ln_ps", bufs=1, space="PSUM") as lps, \
                 tc.tile_pool(name="ln_sb", bufs=2) as lsb:
                sums = lps.tile([1, SL], F32, tag="st", bufs=2)
                sumsq = lps.tile([1, SL], F32, tag="st", bufs=2)
                sq_ = [lsb.tile([128, SL], BF16, tag="lnsq", bufs=2,
                                name=f"sq{t}") for t in range(ND)]
                for t in range(ND):
                    nc.gpsimd.tensor_tensor(sq_[t][:], src_bf[t][:],
                                            src_bf[t][:], op=ALU.mult)
                for t in range(ND):
                    nc.tensor.matmul(sums[:], ones_colb[:], src_bf[t][:],
                                     start=(t == 0), stop=(t == ND - 1))
                for t in range(ND):
                    nc.tensor.matmul(sumsq[:], ones_colb[:], sq_[t][:],
                                     start=(t == 0), stop=(t == ND - 1))
                mean = lsb.tile([1, SL], F32)
                ex2 = lsb.tile([1, SL], F32)
                msq = lsb.tile([1, SL], F32)
                var = lsb.tile([1, SL], F32)
                lnv = lsb.tile([1, SL], F32)
                meanb = lsb.tile([1, SL], BF16)
                rstdb = lsb.tile([1, SL], BF16)
                nc.vector.tensor_scalar_mul(mean[:], sums[:], 1.0 / D)
                nc.vector.tensor_scalar_mul(ex2[:], sumsq[:], 1.0 / D)
                nc.vector.tensor_tensor(msq[:], mean[:], mean[:], op=ALU.mult)
                nc.vector.tensor_tensor(var[:], ex2[:], msq[:], op=ALU.subtract)
                nc.scalar.activation(lnv[:], var[:], AF.Ln, bias=eps_tile[:])
                nc.scalar.activation(rstdb[:], lnv[:], AF.Exp, scale=-0.5)
                nc.vector.tensor_copy(meanb[:], mean[:])
                with tc.tile_pool(name="lnb_ps", bufs=1, space="PSUM") as bps:
                    meanB = bps.tile([128, SL], F32, tag="bc", bufs=2)
                    rstdB = bps.tile([128, SL], F32, tag="bc", bufs=2)
                    nc.tensor.matmul(meanB[:], ones128b[:], meanb[:],
                                     start=True, stop=True)
                    nc.tensor.matmul(rstdB[:], ones128b[:], rstdb[:],
                                     start=True, stop=True)
                    # gpsimd can't read PSUM: stage broadcasts in SBUF
                    meanS = lsb.tile([128, SL], F32)
                    rstdS = lsb.tile([128, SL], F32)
                    nc.vector.tensor_copy(meanS[:], meanB[:])
                    nc.vector.tensor_copy(rstdS[:], rstdB[:])
                    for t in range(ND):
                        cen = lsb.tile([128, SL], F32, tag="lncen", bufs=2,
                                       name=f"cen{t}")
                        nc.gpsimd.tensor_tensor(cen[:], src_f32[t][:],
                                                meanS[:], op=ALU.subtract)
                        nc.gpsimd.tensor_tensor(cen[:], cen[:], rstdS[:],
                                                op=ALU.mult)
                        nc.scalar.activation(out_f32[t][:], cen[:], AF.Identity,
                                             bias=b_sb[:, t:t + 1],
                                             scale=w_sb[:, t:t + 1])
                        nc.gpsimd.tensor_scalar(out_q[:, SL * t:SL * (t + 1)],
                                                cen[:], wq_sb[:, t:t + 1],
                                                bq_sb[:, t:t + 1],
                                                op0=ALU.mult, op1=ALU.add)

        # ==== P0: LN1 =======================================================
        ln1x8 = persist.tile([128, ND * SL], FP8)
        with tc.tile_pool(name="xin", bufs=1) as xin:
            x_sb = [xin.tile([128, SL], F32, name=f"x_sb{t}") for t in range(ND)]
            x_bf = [xin.tile([128, SL], BF16, name=f"x_bf{t}") for t in range(ND)]
            for t in range(ND):
                nc.sync.dma_start(x_sb[t][:], xT[128 * t:128 * (t + 1), :])
                nc.gpsimd.dma_start(x_bf[t][:], xT[128 * t:128 * (t + 1), :])
            layernorm_T(x_sb, x_bf, ln1w_sb, ln1b_sb, ln1w8, ln1b8, FP8,
                        ln1x, ln1x8)

        # ==== P1: k,v (chunked by head group) + allgather; then q ===========
        kv_own = [dram.tile([128 * 2048], FP8, name=f"kv_own{g}")
                  for g in range(NG)]
        gspace = "Shared" if NC > 4 else "Local"
        kv_gath = [dram.tile([NC * 128 * 2048], FP8, addr_space=gspace,
                             name=f"kv_gath{g}") for g in range(NG)]
        grp = [list(range(NC))]

        def ln1x8_pair(dp, lo, n):
            """AP [128, 2, n] pairing d-tiles (dp, dp+3), cols lo:lo+n."""
            return ln1x8.rearrange("p (two c) -> p two c", two=2)[
                :, :, SL * dp + lo:SL * dp + lo + n]

        with tc.tile_pool(name="wqkv", bufs=1) as wp, \
             tc.tile_pool(name="qkv_ps", bufs=1, space="PSUM") as qps, \
             tc.tile_pool(name="kv_sb", bufs=1) as kvp:
            w8 = [wp.tile([128, 2 * 2304], FP8, name=f"w8_{dp}")
                  for dp in range(NDP)]
            for dp in range(NDP):
                nc.sync.dma_start(w8[dp][:],
                                  w_qkv8[128 * dp:128 * (dp + 1), :])

            def wq_ap(dp, base, n):
                return w8[dp].rearrange("p (two c) -> p two c", two=2)[
                    :, :, base:base + n]

            # bvb64: v-bias broadcast to [128, D], x32
            bvb64 = kvp.tile([128, D], F32)
            with tc.tile_pool(name="bv_ps", bufs=1, space="PSUM") as bvp:
                for i in range(2):
                    bvb = bvp.tile([128, 512], F32, tag="bvb", bufs=2,
                                   name=f"bvb{i}")
                    nc.tensor.matmul(bvb[:, 0:384], ones_row32[:],
                                     b_v_sb[:, 384 * i:384 * (i + 1)],
                                     start=True, stop=True)
                    nc.vector.tensor_copy(bvb64[:, 384 * i:384 * (i + 1)],
                                          bvb[:, 0:384])

            for g in range(NG):
                kown = kv_own[g].rearrange("(p c) -> p c", p=128)
                for i in range(2):
                    ps = qps.tile([128, SL], F32, tag="qk", bufs=3)
                    for dp in range(NDP):
                        nc.tensor.matmul(
                            ps[:], wq_ap(dp, 768 + 256 * g + 128 * i, 128),
                            ln1x8_pair(dp, 0, SL), start=(dp == 0),
                            stop=(dp == NDP - 1), perf_mode=DR)
                    k8t = kvp.tile([128, SL], FP8, tag="k8", bufs=2,
                                   name=f"k8_{g}_{i}")
                    nc.scalar.activation(
                        k8t[:], ps[:], AF.Identity,
                        bias=b_k64_sb[:, 2 * g + i:2 * g + i + 1])
                    nc.sync.dma_start(kown[:, 512 * i:512 * (i + 1)], k8t[:])
                for m in range(4):
                    ps = qps.tile([128, 512], F32, tag="v", bufs=3)
                    pv = ps[:, 0:256]
                    for dp in range(NDP):
                        nc.tensor.matmul(
                            pv, ln1x8_pair(dp, 128 * m, 128),
                            wq_ap(dp, 1536 + 256 * g, 256),
                            start=(dp == 0), stop=(dp == NDP - 1), perf_mode=DR)
                    v8t = kvp.tile([128, 256], FP8, tag="v8", bufs=3,
                                   name=f"v8_{g}_{m}")
                    nc.vector.scalar_tensor_tensor(
                        v8t[:], pv, 1.0, bvb64[:, 256 * g:256 * (g + 1)],
                        op0=ALU.mult, op1=ALU.add)
                    nc.sync.dma_start(
                        kown[:, 1024 + 256 * m:1024 + 256 * (m + 1)], v8t[:])
                nc.gpsimd.collective_compute(
                    "AllGather", ALU.bypass, replica_groups=grp,
                    ins=[kv_own[g][:]], outs=[kv_gath[g][:]])

            for g in range(NG):
                for i in range(2):
                    ps = qps.tile([128, SL], F32, tag="qk", bufs=3)
                    for dp in range(NDP):
                        nc.tensor.matmul(
                            ps[:], wq_ap(dp, 256 * g + 128 * i, 128),
                            ln1x8_pair(dp, 0, SL), start=(dp == 0),
                            stop=(dp == NDP - 1), perf_mode=DR)
                    nc.scalar.activation(
                        q8[g][:, SL * i:SL * (i + 1)], ps[:], AF.Identity,
                        bias=b_q64_sb[:, 2 * g + i:2 * g + i + 1])

        # ==== P2+P3: attention ==============================================
        # prefetch FFN weights during attention (pool outlives attn)
        wffn = tc.alloc_tile_pool(name="wffn", bufs=1)
        w_fc_sb = [wffn.tile([128, 4 * D], BF16, name=f"wfc{t}")
                   for t in range(ND)]
        for t in range(ND):
            nc.sync.dma_start(w_fc_sb[t][:], w_fcT[128 * t:128 * (t + 1), :])
        w_pj_sb = [wffn.tile([128, D], BF16, name=f"wpj{m}") for m in range(NFF)]
        for m in range(NFF):
            nc.sync.dma_start(w_pj_sb[m][:], w_projT[128 * m:128 * (m + 1), :])
        attn = tc.alloc_tile_pool(name="attn", bufs=1)

        kT = [attn.tile([128, 2 * S], FP8, name=f"kT{g}") for g in range(NG)]
        # v_all: per head h, key-tile b: cols h*2048 + b*64
        v_all = attn.tile([128, H * NSK * 64], FP8)
        for g in range(NG):
            gath = kv_gath[g].rearrange("(c p w) -> c p w", c=NC, p=128)
            kdst = kT[g].rearrange("p (two c s) -> p two c s", two=2, c=NC)
            for c in range(NC):
                nc.sync.dma_start(
                    kdst[:, :, c, :],
                    gath[c, :, 0:1024].rearrange("p (two s) -> p two s", two=2))
            for c in range(NC):
                for m in range(4):
                    b = 4 * c + m
                    vdst = v_all.rearrange(
                        "p (h bb d) -> p h bb d", h=H, bb=NSK)[
                        :, 4 * g:4 * g + 4, b, :]
                    vsrc = gath[c, :, 1024 + 256 * m:1024 + 256 * (m + 1)
                                ].rearrange("p (h d) -> p h d", h=4)
                    nc.sync.dma_start(vdst, vsrc)

        if dbg_dump:
            nc.sync.dma_start(d_ln1x[:], ln1x[0][:])
            nc.sync.dma_start(d_ln1x8[:], ln1x8[:])
            nc.sync.dma_start(d_q8[:], q8[0][:])
            nc.sync.dma_start(d_kT[:], kT[0][:])
            nc.sync.dma_start(d_vall[:], v_all[:])
        ones2_f8 = persist.tile([128, 32], FP8)
        nc.vector.memset(ones2_f8[:], 1.0)
        ones64f = persist.tile([1, 64], F32)
        nc.vector.memset(ones64f[:], 1.0 / 32.0)

        with tc.tile_pool(name="sg_ps", bufs=1, space="PSUM") as sps, \
             tc.tile_pool(name="cs_ps", bufs=1, space="PSUM") as cps, \
             tc.tile_pool(name="exp_sb", bufs=1) as epool:
            vv = v_all.rearrange("p (hb two m) -> p hb two m", two=2, m=64)
            for h in range(H):
                g, j = h // 4, h % 4
                ctx = cps.tile([64, SL], F32, tag="ctx", bufs=1)
                sd = cps.tile([1, SL], F32, tag="sd", bufs=1)
                kTg = kT[g].rearrange("p (two s) -> p two s", two=2)
                q8g = q8[g].rearrange("p (two s) -> p two s", two=2)
                for p_ in range(NPAIR):
                    sg = sps.tile([128, 2 * SL], F32, tag="sg", bufs=3)
                    for i in range(2):
                        b = 2 * p_ + i
                        nc.tensor.matmul(
                            sg[:, SL * i:SL * (i + 1)],
                            kTg[32 * j:32 * (j + 1), :, 128 * b:128 * (b + 1)],
                            q8g[32 * j:32 * (j + 1), :, :],
                            start=True, stop=True, perf_mode=DR,
                            tile_position=(32 * j, 0))
                    ex = epool.tile([128, 2 * SL], FP8, tag="ex", bufs=3)
                    if (h * NPAIR + p_) % 2 == 0:
                        nc.scalar.activation(ex[:], sg[:], AF.Exp,
                                             scale=cfg.yscale, bias=lnlam[:])
                    else:
                        nc.vector._custom_dve(EXP_POLY, out=ex[:], in0=sg[:],
                                              s0=EC0 * cfg.yscale,
                                              s1=EC1, imm2=EC2)
                    if dbg_dump and h == 0 and p_ == 0:
                        nc.sync.dma_start(d_ex[:], ex[:])
                    exr = ex.rearrange("p (two s) -> p two s", two=2)
                    nc.tensor.matmul(ctx[:], vv[:, h * NPAIR + p_, :, :],
                                     exr[:],
                                     start=(p_ == 0), stop=(p_ == NPAIR - 1),
                                     perf_mode=DR)
                    nc.tensor.matmul(sd[:],
                                     ones2_f8.rearrange("p (two m) -> p two m",
                                                        two=2, m=16)[:, :, 0:1],
                                     exr[:],
                                     start=(p_ == 0), stop=(p_ == NPAIR - 1),
                                     perf_mode=DR)
                # epilogue: r = 1/(32*den) (x32 from v8 scaling, folded into
                # the ones64f broadcast); x2 = ln1x + ctx*r. Odd heads land on
                # partitions 64:128 via a partition-shift DMA.
                t, half = h // 2, h % 2
                ctxS = epool.tile([64, SL], F32, tag="ctxS", bufs=2)
                nc.vector.tensor_copy(ctxS[:], ctx[:])
                den_r = epool.tile([1, SL], F32, tag="den_r", bufs=2)
                nc.vector.reciprocal_approx_fast(den_r[:], sd[:])
                rb = sps.tile([128, 2 * SL], F32, tag="sg", bufs=3)
                nc.tensor.matmul(rb[0:64, 0:SL], ones64f[:], den_r[:],
                                 start=True, stop=True)
                if dbg_dump and h == 0:
                    nc.sync.dma_start(d_ctxS[:], ctxS[:])
                    nc.sync.dma_start(d_denr[:], den_r[:])
                cn = epool.tile([64, SL], F32, tag="cn", bufs=2)
                nc.vector.scalar_tensor_tensor(cn[:], rb[0:64, 0:SL], 1.0,
                                               ctxS[:],
                                               op0=ALU.mult, op1=ALU.mult)
                if half == 0:
                    nc.gpsimd.tensor_tensor(x2[t][0:64, :], cn[:],
                                            ln1x[t][0:64, :], op=ALU.add)
                else:
                    nc.sync.dma_start(x2[t][64:128, :], cn[:])
                    nc.gpsimd.tensor_tensor(x2[t][64:128, :],
                                            x2[t][64:128, :],
                                            ln1x[t][64:128, :], op=ALU.add)
        if dbg_dump:
            nc.sync.dma_start(d_x2[:], x2[0][:])
        attn.release()

        # ==== P4+P5: LN2 + FFN =============================================
        with tc.tile_pool(name="ffn_sb", bufs=1) as fp:
            x2bf = [fp.tile([128, SL], BF16, name=f"x2bf{t}") for t in range(ND)]
            for t in range(ND):
                nc.gpsimd.dma_start(x2bf[t][:], x2[t][:])
            x2ln = ln1x   # reuse
            x2lnb = fp.tile([128, ND * SL], BF16)
            layernorm_T(x2, x2bf, ln2w_sb, ln2b_sb, ln2w_sb, ln2b_sb, BF16,
                        x2ln, x2lnb)
            fps = tc.alloc_tile_pool(name="ffn_ps", bufs=1, space="PSUM")
            h_sb = fp.tile([128, NFF * SL], BF16)
            for m in range(NFF):
                ps = fps.tile([128, SL], F32, tag="h", bufs=4)
                for t in range(ND):
                    nc.tensor.matmul(ps[:], w_fc_sb[t][:, 128 * m:128 * (m + 1)],
                                     x2lnb[:, SL * t:SL * (t + 1)],
                                     start=(t == 0), stop=(t == ND - 1))
                nc.scalar.activation(h_sb[:, SL * m:SL * (m + 1)], ps[:],
                                     AF.Gelu_apprx_tanh,
                                     bias=b_fc_sb[:, m:m + 1])
            for t in range(ND):
                ps = fps.tile([128, SL], F32, tag="o", bufs=2)
                for m in range(NFF):
                    nc.tensor.matmul(ps[:], w_pj_sb[m][:, 128 * t:128 * (t + 1)],
                                     h_sb[:, SL * m:SL * (m + 1)],
                                     start=(m == 0), stop=(m == NFF - 1))
                o = fp.tile([128, SL], F32, tag="out", bufs=2, name=f"o{t}")
                nc.vector.scalar_tensor_tensor(o[:], ps[:],
                                               b_proj_sb[:, t:t + 1],
                                               x2ln[t][:],
                                               op0=ALU.add, op1=ALU.add)
                nc.sync.dma_start(outT[128 * t:128 * (t + 1), :], o[:])
            fps.release()
        wffn.release()

    nc.compile()
    return nc


# ---- host side --------------------------------------------------------------

def _prep_inputs(cfg, x, ln1_w, ln1_b, w_attn, b_attn, ln2_w, ln2_b,
                 w_fc, b_fc, w_proj, b_proj):
    D, H, NC, SL, ND, NDP, NFF, NG = (cfg.D, cfg.H, cfg.NC, cfg.SL, cfg.ND,
                                      cfg.NDP, cfg.NFF, cfg.NG)
    import ml_dtypes
    bf16 = ml_dtypes.bfloat16
    fp8 = ml_dtypes.float8_e4m3

    def pp(v, n):
        return np.ascontiguousarray(v.reshape(n, 128).T.astype(np.float32))

    # column perms: block (g, i) covers cols 256g+128i..+128; col p ->
    # head 4g + p//32, dim 32*i + p%32.
    qcols = np.empty(768, np.int64)
    kcols = np.empty(768, np.int64)
    for g in range(NG):
        for i in range(2):
            base = 256 * g + 128 * i
            for p in range(128):
                h = 4 * g + p // 32
                d = 32 * i + p % 32
                qcols[base + p] = h * 64 + d
                kcols[base + p] = D + h * 64 + d
    vcols = np.arange(2 * D, 3 * D)
    cols = np.concatenate([qcols, kcols, vcols])          # [2304]
    wsel = w_attn[cols, :].T * 4.0    # x4: with x8 activations -> 32x outputs
    # (TRN fp8e4 saturates at 240; 64x q/k would overflow)
    # pair layout [NDP, 128, 2, 2304]: row pairs are d-tiles (dp, dp+3)
    wp8 = np.empty((NDP, 128, 2, 2304), np.float32)
    for dp in range(NDP):
        for j in range(2):
            t = dp + 3 * j
            wp8[dp, :, j, :] = wsel[128 * t:128 * (t + 1), :]
    w_qkv8 = np.ascontiguousarray(
        wp8.reshape(NDP * 128, 2 * 2304).astype(fp8))

    b_q64 = np.ascontiguousarray(
        (b_attn[qcols] * 32.0).reshape(ND, 128).T.astype(np.float32))
    b_k64 = np.ascontiguousarray(
        (b_attn[kcols] * 32.0).reshape(ND, 128).T.astype(np.float32))
    b_v = np.ascontiguousarray(b_attn[2 * D:].reshape(1, D).astype(np.float32))

    common = {
        "w_qkv8": w_qkv8,
        "b_q64": b_q64, "b_k64": b_k64, "b_v": b_v,
        "ln1w": pp(ln1_w, ND), "ln1b": pp(ln1_b, ND),
        "ln2w": pp(ln2_w, ND), "ln2b": pp(ln2_b, ND),
        "w_fcT": np.ascontiguousarray(w_fc.T.astype(bf16)),
        "b_fc": pp(b_fc, NFF),
        "w_projT": np.ascontiguousarray(w_proj.T.astype(bf16)),
        "b_proj": pp(b_proj, ND),
    }
    xT = np.ascontiguousarray(x.T.astype(np.float32))
    in_maps = []
    for c in range(NC):
        m = dict(common)
        m["xT"] = np.ascontiguousarray(xT[:, c * SL:(c + 1) * SL])
        in_maps.append(m)
    return in_maps


_CACHE = {}


def kernel(**inputs):
    cfg = Cfg()
    inputs = {k: np.asarray(v) for k, v in inputs.items()}
    in_maps = _prep_inputs(cfg, **inputs)
    if "nc" not in _CACHE:
        _CACHE["nc"] = build(cfg)
    nc = _CACHE["nc"]
    from concourse.bass_utils import run_bass_kernel_spmd
    res = run_bass_kernel_spmd(nc, in_maps, list(range(cfg.NC)))
    outs = [np.asarray(res.results[c]["outT"], dtype=np.float32).T
            for c in range(cfg.NC)]
    return np.ascontiguousarray(np.concatenate(outs, axis=0))


# revision 20
# speedup vs baseline: 1.3219x; 1.3219x over previous
"""Trainium2 Bass kernel for nn_DecoderBlock (S=4096, D=768, H=12).

Strategy (8 NeuronCores, SPMD, sequence-parallel: core c owns rows
[c*512, (c+1)*512)):

  - All activations transposed (features on partitions, sequence free).
    LayerNorm stats via ones-vector matmuls on bf16 copies (1 cyc/row).
  - Attention runs in fp8(e4m3, TRN flavor: max 240):
      * scores: per head-pair tile (head A dims on partitions 0:64, head
        B on 64:128), one K=64 fp8 matmul per (head, key-tile) with
        tile positions ping-ponging (0,0)/(64,0) -- measured 108ns per
        [128,512] matmul (2x over a fixed position).
      * PV: DoubleRow pairs two key-tiles per matmul (effective K=256 in
        one 216ns pass); the stationary V blocks are padded to 80 cols
        with an all-ones column at 64, so the softmax denominator
        accumulates on PSUM partition 64 for free.
  - Softmax exp splits across TWO engines: the Scalar engine computes
    lam*exp(y) (bias=ln lam) for even key-tiles while the Vector engine
    computes the same for odd tiles via a custom 6-stage DVE op
    (((c0*y+c1)^2+c2)^2)^2 ~= lam*exp(y) (+-2.8% shape error; the
    common lam cancels in the softmax normalization).
  - x is pre-scaled x8 and qkv weights x4 (TRN fp8 max is 240; q/k/v
    then sit at 32x true value, absmax ~130). The 1/32 folds into the
    denominator reciprocal broadcast; 1/(32*32) and softmax 1/sqrt(64)
    fold into the exp input scale.
  - K and V are cast to fp8 and exchanged with 3 chunked AllGathers
    (one per 4-head group) so attention starts while later chunks fly.
  - QKV projections use fp8 DoubleRow over d-tile pairs (half the
    matmul count); FFN stays bf16 (fp8 would blow the 2e-2 budget);
    FFN weights are prefetched during attention.
"""

import os
import sys

for _p in ("/opt/trn_rl_repo", os.path.expanduser("~/.axon_site/_ro/trn_rl_repo")):
    if os.path.isdir(_p) and _p not in sys.path:
        sys.path.append(_p)

import numpy as np
from contextlib import ExitStack

import concourse.bass as bass
import concourse.tile as tile
from concourse import bacc, mybir
import concourse.dve_ops as dve_ops
from concourse.dve_spec import Spec, Src0, C0, C1, C2, sq, lower as dve_lower
from concourse.dve_uop import DveOpSpec

F32 = mybir.dt.float32
BF16 = mybir.dt.bfloat16
FP8 = mybir.dt.float8e4
AF = mybir.ActivationFunctionType
ALU = mybir.AluOpType
DR = mybir.MatmulPerfMode.DoubleRow

# exp-poly coefficients: ((EC0*y+EC1)^2+EC2)^4 ~= ELAM * e^y on |y|<=2.2
EC0, EC1, EC2 = 0.19508098, 0.82453421, 0.56463811
ELAM = 2.3759038641914842


def _ref_exp_poly(in0, in1, s0, s1, imm2):
    t = (in0.astype(np.float32) * s0 + s1) ** 2 + imm2
    return (t * t) ** 2


def register_exp_poly():
    for op in dve_ops.OPS:
        if op.name == "EXP_POLY_ANT":
            return op
    spec = Spec(body=sq(sq(sq(Src0 * C0 + C1) + C2)), reference=_ref_exp_poly)
    name = "EXP_POLY_ANT"
    opcode = dve_ops._CUSTOM_DVE_ROW_BASE + len(dve_ops.OPS)
    shas = {}
    for ver in ("v3", "v4"):
        try:
            s = DveOpSpec(name=name, opcode=opcode,
                          uops=dve_lower(spec, ver=ver), rd1_en=False)
            shas[ver] = s.sha(ver)
        except Exception:
            pass
    op = dve_ops.DveOp(name, spec, subdim=False, uops_sha=shas)
    dve_ops.OPS.append(op)
    dve_ops.CUSTOM_DVE_SPECS[name] = op.spec
    dve_ops._SUB_OPCODE_FOR_NAME[name] = opcode
    return op


EXP_POLY = register_exp_poly()


class Cfg:
    def __init__(self, S=4096, D=768, H=12, NC=8, eps=1e-5):
        self.S, self.D, self.H, self.NC, self.eps = S, D, H, NC, eps
        self.DH = D // H              # 64
        self.SL = S // NC             # 512 local rows
        self.ND = D // 128            # 6 d-tiles
        self.NDP = self.ND // 2       # 3 d-tile pairs
        self.NSK = S // 128           # 32 key tiles
        self.NPAIR = self.NSK // 2    # 16 key-tile pairs
        self.NG = 3                   # head groups (4 heads each)
        self.NHP = H // 2             # 6 head pairs
        self.NFF = 4 * D // 128       # 24 ffn tiles
        self.scale = 1.0 / float(np.sqrt(self.DH))
        self.yscale = self.scale / 1024.0   # q8,k8 are 32x the true values


def build(cfg: Cfg, debug=False, enable_asserts=False, dbg_dump=False):
    nc = bacc.Bacc(
        "TRN2",
        target_bir_lowering=False,
        debug=debug,
        enable_asserts=enable_asserts,
        num_devices=cfg.NC,
    )
    S, D, H, SL = cfg.S, cfg.D, cfg.H, cfg.SL
    ND, NDP, NSK, NPAIR, NG, NHP, NFF, NC = (
        cfg.ND, cfg.NDP, cfg.NSK, cfg.NPAIR, cfg.NG, cfg.NHP, cfg.NFF, cfg.NC)

    # ---- DRAM I/O ----------------------------------------------------------
    xT = nc.dram_tensor("xT", [D, SL], F32, kind="ExternalInput").ap()
    w_qkv8 = nc.dram_tensor("w_qkv8", [NDP * 128, 2 * 2304], FP8,
                            kind="ExternalInput").ap()
    b_q32 = nc.dram_tensor("b_q32", [128, ND], F32, kind="ExternalInput").ap()
    b_k32 = nc.dram_tensor("b_k32", [128, ND], F32, kind="ExternalInput").ap()
    b_v = nc.dram_tensor("b_v", [1, D], F32, kind="ExternalInput").ap()
    ln1w = nc.dram_tensor("ln1w", [128, ND], F32, kind="ExternalInput").ap()
    ln1b = nc.dram_tensor("ln1b", [128, ND], F32, kind="ExternalInput").ap()
    ln2w = nc.dram_tensor("ln2w", [128, ND], F32, kind="ExternalInput").ap()
    ln2b = nc.dram_tensor("ln2b", [128, ND], F32, kind="ExternalInput").ap()
    w_fcT = nc.dram_tensor("w_fcT", [D, 4 * D], BF16, kind="ExternalInput").ap()
    b_fc = nc.dram_tensor("b_fc", [128, NFF], F32, kind="ExternalInput").ap()
    w_projT = nc.dram_tensor("w_projT", [4 * D, D], BF16,
                             kind="ExternalInput").ap()
    b_proj = nc.dram_tensor("b_proj", [128, ND], F32, kind="ExternalInput").ap()
    outT = nc.dram_tensor("outT", [D, SL], F32, kind="ExternalOutput").ap()
    if dbg_dump:
        d_q8 = nc.dram_tensor("d_q8", [128, SL], FP8, kind="ExternalOutput").ap()
        d_kT = nc.dram_tensor("d_kT", [128, S], FP8, kind="ExternalOutput").ap()
        d_vall = nc.dram_tensor("d_vall", [128, H * (S // 256) * 160], FP8,
                                kind="ExternalOutput").ap()
        d_ex = nc.dram_tensor("d_ex", [128, 4 * SL], FP8, kind="ExternalOutput").ap()
        d_ctxS = nc.dram_tensor("d_ctxS", [64, SL], F32, kind="ExternalOutput").ap()
        d_den = nc.dram_tensor("d_den", [1, SL], F32, kind="ExternalOutput").ap()
        d_x2 = nc.dram_tensor("d_x2", [128, SL], F32, kind="ExternalOutput").ap()

    with tile.TileContext(nc) as tc, ExitStack() as top:
        persist = top.enter_context(tc.tile_pool(name="persist", bufs=1))
        dram = top.enter_context(tc.tile_pool(name="dram", bufs=1, space="DRAM"))

        ones_colb = persist.tile([128, 1], BF16)
        nc.vector.memset(ones_colb[:], 1.0)
        ones_row32 = persist.tile([1, 128], F32)
        nc.vector.memset(ones_row32[:], 32.0)
        ones128b = persist.tile([1, 128], BF16)
        nc.vector.memset(ones128b[:], 1.0)
        eps_tile = persist.tile([1, 1], F32)
        nc.vector.memset(eps_tile[:], float(cfg.eps))
        lnlam = persist.tile([128, 1], F32)
        nc.vector.memset(lnlam[:], float(np.log(ELAM)))
        ones64f = persist.tile([1, 64], F32)
        nc.vector.memset(ones64f[:], 1.0 / 32.0)

        b_q32_sb = persist.tile([128, ND], F32)
        nc.sync.dma_start(b_q32_sb[:], b_q32[:])
        b_k32_sb = persist.tile([128, ND], F32)
        nc.sync.dma_start(b_k32_sb[:], b_k32[:])
        b_v_sb = persist.tile([1, D], F32)
        nc.sync.dma_start(b_v_sb[:], b_v[:])
        ln1w_sb = persist.tile([128, ND], F32)
        nc.sync.dma_start(ln1w_sb[:], ln1w[:])
        ln1b_sb = persist.tile([128, ND], F32)
        nc.sync.dma_start(ln1b_sb[:], ln1b[:])
        ln2w_sb = persist.tile([128, ND], F32)
        nc.sync.dma_start(ln2w_sb[:], ln2w[:])
        ln2b_sb = persist.tile([128, ND], F32)
        nc.sync.dma_start(ln2b_sb[:], ln2b[:])
        b_fc_sb = persist.tile([128, NFF], F32)
        nc.sync.dma_start(b_fc_sb[:], b_fc[:])
        b_proj_sb = persist.tile([128, ND], F32)
        nc.sync.dma_start(b_proj_sb[:], b_proj[:])

        ln1w8 = persist.tile([128, ND], F32)
        nc.vector.tensor_scalar_mul(ln1w8[:], ln1w_sb[:], 8.0)
        ln1b8 = persist.tile([128, ND], F32)
        nc.vector.tensor_scalar_mul(ln1b8[:], ln1b_sb[:], 8.0)

        # persistent activations
        ln1x = [persist.tile([128, SL], F32, name=f"ln1x{t}") for t in range(ND)]
        x2 = [persist.tile([128, SL], F32, name=f"x2_{t}") for t in range(ND)]
        q8 = [persist.tile([128, SL], FP8, name=f"q8_{hp}") for hp in range(NHP)]

        def layernorm_T(src_f32, src_bf, w_sb, b_sb, wq_sb, bq_sb, q_dtype,
                        out_f32, out_q):
            """LN over the partition (feature) axis; stats on bf16 copies."""
            with tc.tile_pool(name="ln_ps", bufs=1, space="PSUM") as lps, \
                 tc.tile_pool(name="ln_sb", bufs=2) as lsb:
                sums = lps.tile([1, SL], F32, tag="st", bufs=2)
                sumsq = lps.tile([1, SL], F32, tag="st", bufs=2)
                sq_ = [lsb.tile([128, SL], BF16, tag="lnsq", bufs=2,
                                name=f"sq{t}") for t in range(ND)]
                for t in range(ND):
                    nc.gpsimd.tensor_tensor(sq_[t][:], src_bf[t][:],
                                            src_bf[t][:], op=ALU.mult)
                for t in range(ND):
                    nc.tensor.matmul(sums[:], ones_colb[:], src_bf[t][:],
                                     start=(t == 0), stop=(t == ND - 1))
                for t in range(ND):
                    nc.tensor.matmul(sumsq[:], ones_colb[:], sq_[t][:],
                                     start=(t == 0), stop=(t == ND - 1))
                mean = lsb.tile([1, SL], F32)
                ex2 = lsb.tile([1, SL], F32)
                msq = lsb.tile([1, SL], F32)
                var = lsb.tile([1, SL], F32)
                lnv = lsb.tile([1, SL], F32)
                meanb = lsb.tile([1, SL], BF16)
                rstdb = lsb.tile([1, SL], BF16)
                nc.vector.tensor_scalar_mul(mean[:], sums[:], 1.0 / D)
                nc.vector.tensor_scalar_mul(ex2[:], sumsq[:], 1.0 / D)
                nc.vector.tensor_tensor(msq[:], mean[:], mean[:], op=ALU.mult)
                nc.vector.tensor_tensor(var[:], ex2[:], msq[:], op=ALU.subtract)
                nc.scalar.activation(lnv[:], var[:], AF.Ln, bias=eps_tile[:])
                nc.scalar.activation(rstdb[:], lnv[:], AF.Exp, scale=-0.5)
                nc.vector.tensor_copy(meanb[:], mean[:])
                with tc.tile_pool(name="lnb_ps", bufs=1, space="PSUM") as bps:
                    meanB = bps.tile([128, SL], F32, tag="bc", bufs=2)
                    rstdB = bps.tile([128, SL], F32, tag="bc", bufs=2)
                    nc.tensor.matmul(meanB[:], ones128b[:], meanb[:],
                                     start=True, stop=True)
                    nc.tensor.matmul(rstdB[:], ones128b[:], rstdb[:],
                                     start=True, stop=True)
                    # gpsimd can't read PSUM: stage broadcasts in SBUF
                    meanS = lsb.tile([128, SL], F32)
                    rstdS = lsb.tile([128, SL], F32)
                    nc.vector.tensor_copy(meanS[:], meanB[:])
                    nc.vector.tensor_copy(rstdS[:], rstdB[:])
                    for t in range(ND):
                        cen = lsb.tile([128, SL], F32, tag="lncen", bufs=2,
                                       name=f"cen{t}")
                        nc.gpsimd.tensor_tensor(cen[:], src_f32[t][:],
                                                meanS[:], op=ALU.subtract)
                        nc.gpsimd.tensor_tensor(cen[:], cen[:], rstdS[:],
                                                op=ALU.mult)
                        nc.scalar.activation(out_f32[t][:], cen[:], AF.Identity,
                                             bias=b_sb[:, t:t + 1],
                                             scale=w_sb[:, t:t + 1])
                        nc.gpsimd.tensor_scalar(out_q[:, SL * t:SL * (t + 1)],
                                                cen[:], wq_sb[:, t:t + 1],
                                                bq_sb[:, t:t + 1],
                                                op0=ALU.mult, op1=ALU.add)

        # ==== P0: LN1 =======================================================
        ln1x8 = persist.tile([128, ND * SL], FP8)
        with tc.tile_pool(name="xin", bufs=1) as xin:
            x_sb = [xin.tile([128, SL], F32, name=f"x_sb{t}") for t in range(ND)]
            x_bf = [xin.tile([128, SL], BF16, name=f"x_bf{t}") for t in range(ND)]
            for t in range(ND):
                nc.sync.dma_start(x_sb[t][:], xT[128 * t:128 * (t + 1), :])
                nc.gpsimd.dma_start(x_bf[t][:], xT[128 * t:128 * (t + 1), :])
            layernorm_T(x_sb, x_bf, ln1w_sb, ln1b_sb, ln1w8, ln1b8, FP8,
                        ln1x, ln1x8)

        # ==== P1: k,v (chunked by head group) + allgather; then q ===========
        # kv_own[g]: [128, 2048]: cols 0:1024 k (2 head-pairs), 1024: v
        kv_own = [dram.tile([128 * 2048], FP8, name=f"kv_own{g}")
                  for g in range(NG)]
        gspace = "Shared" if NC > 4 else "Local"
        kv_gath = [dram.tile([NC * 128 * 2048], FP8, addr_space=gspace,
                             name=f"kv_gath{g}") for g in range(NG)]
        grp = [list(range(NC))]

        def ln1x8_pair(dp, lo, n):
            """AP [128, 2, n] pairing d-tiles (dp, dp+3), cols lo:lo+n."""
            return ln1x8.rearrange("p (two c) -> p two c", two=2)[
                :, :, SL * dp + lo:SL * dp + lo + n]

        with tc.tile_pool(name="wqkv", bufs=1) as wp, \
             tc.tile_pool(name="qkv_ps", bufs=1, space="PSUM") as qps, \
             tc.tile_pool(name="kv_sb", bufs=1) as kvp:
            w8 = [wp.tile([128, 2 * 2304], FP8, name=f"w8_{dp}")
                  for dp in range(NDP)]
            for dp in range(NDP):
                nc.sync.dma_start(w8[dp][:],
                                  w_qkv8[128 * dp:128 * (dp + 1), :])

            def wq_ap(dp, base, n):
                return w8[dp].rearrange("p (two c) -> p two c", two=2)[
                    :, :, base:base + n]

            # bvb32: v-bias broadcast to [128, D], x32
            bvb32 = kvp.tile([128, D], F32)
            with tc.tile_pool(name="bv_ps", bufs=1, space="PSUM") as bvp:
                for i in range(2):
                    bvb = bvp.tile([128, 512], F32, tag="bvb", bufs=2,
                                   name=f"bvb{i}")
                    nc.tensor.matmul(bvb[:, 0:384], ones_row32[:],
                                     b_v_sb[:, 384 * i:384 * (i + 1)],
                                     start=True, stop=True)
                    nc.vector.tensor_copy(bvb32[:, 384 * i:384 * (i + 1)],
                                          bvb[:, 0:384])

            for g in range(NG):
                kown = kv_own[g].rearrange("(p c) -> p c", p=128)
                for i in range(2):         # head pair 2g+i
                    hp = 2 * g + i
                    ps = qps.tile([128, SL], F32, tag="qk", bufs=3)
                    for dp in range(NDP):
                        nc.tensor.matmul(
                            ps[:], wq_ap(dp, 768 + 128 * hp, 128),
                            ln1x8_pair(dp, 0, SL), start=(dp == 0),
                            stop=(dp == NDP - 1), perf_mode=DR)
                    k8t = kvp.tile([128, SL], FP8, tag="k8", bufs=2,
                                   name=f"k8_{g}_{i}")
                    nc.scalar.activation(
                        k8t[:], ps[:], AF.Identity,
                        bias=b_k32_sb[:, hp:hp + 1])
                    nc.sync.dma_start(kown[:, 512 * i:512 * (i + 1)], k8t[:])
                for m in range(4):
                    ps = qps.tile([128, 512], F32, tag="v", bufs=3)
                    pv = ps[:, 0:256]
                    for dp in range(NDP):
                        nc.tensor.matmul(
                            pv, ln1x8_pair(dp, 128 * m, 128),
                            wq_ap(dp, 1536 + 256 * g, 256),
                            start=(dp == 0), stop=(dp == NDP - 1), perf_mode=DR)
                    v8t = kvp.tile([128, 256], FP8, tag="v8", bufs=3,
                                   name=f"v8_{g}_{m}")
                    nc.vector.scalar_tensor_tensor(
                        v8t[:], pv, 1.0, bvb32[:, 256 * g:256 * (g + 1)],
                        op0=ALU.mult, op1=ALU.add)
                    nc.sync.dma_start(
                        kown[:, 1024 + 256 * m:1024 + 256 * (m + 1)], v8t[:])
                nc.gpsimd.collective_compute(
                    "AllGather", ALU.bypass, replica_groups=grp,
                    ins=[kv_own[g][:]], outs=[kv_gath[g][:]])

            for hp in range(NHP):
                ps = qps.tile([128, SL], F32, tag="qk", bufs=3)
                for dp in range(NDP):
                    nc.tensor.matmul(
                        ps[:], wq_ap(dp, 128 * hp, 128),
                        ln1x8_pair(dp, 0, SL), start=(dp == 0),
                        stop=(dp == NDP - 1), perf_mode=DR)
                nc.scalar.activation(
                    q8[hp][:], ps[:], AF.Identity,
                    bias=b_q32_sb[:, hp:hp + 1])

        # ==== P2+P3: attention ==============================================
        # prefetch FFN weights during attention (pool outlives attn)
        wffn = tc.alloc_tile_pool(name="wffn", bufs=1)
        w_fc_sb = [wffn.tile([128, 4 * D], BF16, name=f"wfc{t}")
                   for t in range(ND)]
        for t in range(ND):
            nc.sync.dma_start(w_fc_sb[t][:], w_fcT[128 * t:128 * (t + 1), :])
        w_pj_sb = [wffn.tile([128, D], BF16, name=f"wpj{m}") for m in range(NFF)]
        for m in range(NFF):
            nc.sync.dma_start(w_pj_sb[m][:], w_projT[128 * m:128 * (m + 1), :])
        attn = tc.alloc_tile_pool(name="attn", bufs=1)

        kT = [attn.tile([128, S], FP8, name=f"kT{hp}") for hp in range(NHP)]
        # v_all: per head h, key-pair p, i in {0,1}: 80-col block
        # col = h*2560 + p*160 + i*80 + d ; col d=64 is all-ones
        v_all = attn.tile([128, H * NPAIR * 160], FP8)
        if dbg_dump:
            nc.vector.memset(v_all[:], 0.0)
        nc.vector.memset(
            v_all.rearrange("p (blk w) -> p blk w", w=80)[:, :, 64:65], 1.0)
        for g in range(NG):
            gath = kv_gath[g].rearrange("(c p w) -> c p w", c=NC, p=128)
            for i in range(2):
                hp = 2 * g + i
                kdst = kT[hp].rearrange("p (c s) -> p c s", c=NC)
                for c in range(NC):
                    nc.sync.dma_start(kdst[:, c, :],
                                      gath[c, :, 512 * i:512 * (i + 1)])
            for c in range(NC):
                for m in range(4):
                    b = 4 * c + m
                    p_, i_ = b // 2, b % 2
                    vdst = v_all.rearrange(
                        "p (h pp w) -> p h pp w", h=H, pp=NPAIR)[
                        :, 4 * g:4 * g + 4, p_, 80 * i_:80 * i_ + 64]
                    vsrc = gath[c, :, 1024 + 256 * m:1024 + 256 * (m + 1)
                                ].rearrange("p (h d) -> p h d", h=4)
                    nc.sync.dma_start(vdst, vsrc)

        if dbg_dump:
            nc.sync.dma_start(d_q8[:], q8[0][:])
            nc.sync.dma_start(d_kT[:], kT[0][:])
            nc.sync.dma_start(d_vall[:], v_all[:])
        with tc.tile_pool(name="sg_ps", bufs=1, space="PSUM") as sps, \
             tc.tile_pool(name="cs_ps", bufs=1, space="PSUM") as cps, \
             tc.tile_pool(name="exp_sb", bufs=1) as epool:
            vv = v_all.rearrange("p (hq two w) -> p hq two w", two=2, w=80)
            for hp in range(NHP):
                ha, hb = 2 * hp, 2 * hp + 1
                ctxA = cps.tile([65, SL], F32, tag="ctxA", bufs=1)
                ctxB = cps.tile([65, SL], F32, tag="ctxB", bufs=1)
                for p_ in range(NPAIR):
                    ex = epool.tile([128, 2 * 2 * SL], FP8, tag="ex", bufs=3)
                    for i in range(2):
                        b = 2 * p_ + i
                        sg = sps.tile([128, 2 * SL], F32, tag="sg", bufs=3)
                        nc.tensor.matmul(
                            sg[:, 0:SL],
                            kT[hp][0:64, 128 * b:128 * (b + 1)],
                            q8[hp][0:64, :], start=True, stop=True,
                            tile_position=(0, 0))
                        nc.tensor.matmul(
                            sg[:, SL:2 * SL],
                            kT[hp][64:128, 128 * b:128 * (b + 1)],
                            q8[hp][64:128, :], start=True, stop=True,
                            tile_position=(64, 0))
                        if (hp * NSK + b) % 2 == 0:
                            nc.scalar.activation(
                                ex[:, 1024 * i:1024 * (i + 1)], sg[:], AF.Exp,
                                scale=cfg.yscale, bias=lnlam[:])
                        else:
                            nc.vector._custom_dve(
                                EXP_POLY, out=ex[:, 1024 * i:1024 * (i + 1)],
                                in0=sg[:], s0=EC0 * cfg.yscale,
                                s1=EC1, imm2=EC2)
                    if dbg_dump and hp == 0 and p_ == 0:
                        nc.sync.dma_start(d_ex[:], ex[:])
                    # ex layout: [i(2), head(2), s(512)]
                    exr = ex.rearrange("p (two hq s) -> p two hq s",
                                       two=2, hq=2)
                    nc.tensor.matmul(ctxA[:],
                                     vv[:, ha * NPAIR + p_, :, 0:65],
                                     exr[:, :, 0, :],
                                     start=(p_ == 0), stop=(p_ == NPAIR - 1),
                                     perf_mode=DR)
                    nc.tensor.matmul(ctxB[:],
                                     vv[:, hb * NPAIR + p_, :, 0:65],
                                     exr[:, :, 1, :],
                                     start=(p_ == 0), stop=(p_ == NPAIR - 1),
                                     perf_mode=DR)
                # epilogue: r = 1/(32*den); x2 = ln1x + ctx*r; head B lands on
                # partitions 64:128 via a partition-shift DMA.
                for (half, ctx) in ((0, ctxA), (1, ctxB)):
                    ctxS = epool.tile([64, SL], F32,
                                      tag=f"ctxS{half}", bufs=2)
                    nc.vector.tensor_copy(ctxS[:], ctx[0:64, :])
                    den_s = epool.tile([1, SL], F32,
                                       tag=f"den_s{half}", bufs=2)
                    nc.vector.tensor_copy(den_s[:], ctx[64:65, :])
                    den_r = epool.tile([1, SL], F32,
                                       tag=f"den_r{half}", bufs=2)
                    nc.vector.reciprocal_approx_fast(den_r[:], den_s[:])
                    rb = sps.tile([128, 2 * SL], F32, tag="sg", bufs=3)
                    nc.tensor.matmul(rb[0:64, 0:SL], ones64f[:], den_r[:],
                                     start=True, stop=True)
                    if dbg_dump and hp == 0 and half == 0:
                        nc.sync.dma_start(d_ctxS[:], ctxS[:])
                        nc.sync.dma_start(d_den[:], den_r[:])
                    cn = epool.tile([64, SL], F32, tag=f"cn{half}", bufs=2)
                    nc.vector.scalar_tensor_tensor(cn[:], rb[0:64, 0:SL], 1.0,
                                                   ctxS[:],
                                                   op0=ALU.mult, op1=ALU.mult)
                    if half == 0:
                        nc.gpsimd.tensor_tensor(x2[hp][0:64, :], cn[:],
                                                ln1x[hp][0:64, :], op=ALU.add)
                    else:
                        nc.sync.dma_start(x2[hp][64:128, :], cn[:])
                        nc.gpsimd.tensor_tensor(x2[hp][64:128, :],
                                                x2[hp][64:128, :],
                                                ln1x[hp][64:128, :],
                                                op=ALU.add)
        if dbg_dump:
            nc.sync.dma_start(d_x2[:], x2[0][:])
        attn.release()

        # ==== P4+P5: LN2 + FFN =============================================
        with tc.tile_pool(name="ffn_sb", bufs=1) as fp:
            x2bf = [fp.tile([128, SL], BF16, name=f"x2bf{t}") for t in range(ND)]
            for t in range(ND):
                nc.gpsimd.dma_start(x2bf[t][:], x2[t][:])
            x2ln = ln1x   # reuse
            x2lnb = fp.tile([128, ND * SL], BF16)
            layernorm_T(x2, x2bf, ln2w_sb, ln2b_sb, ln2w_sb, ln2b_sb, BF16,
                        x2ln, x2lnb)
            fps = tc.alloc_tile_pool(name="ffn_ps", bufs=1, space="PSUM")
            h_sb = fp.tile([128, NFF * SL], BF16)
            for m in range(NFF):
                ps = fps.tile([128, SL], F32, tag="h", bufs=4)
                for t in range(ND):
                    nc.tensor.matmul(ps[:], w_fc_sb[t][:, 128 * m:128 * (m + 1)],
                                     x2lnb[:, SL * t:SL * (t + 1)],
                                     start=(t == 0), stop=(t == ND - 1))
                nc.scalar.activation(h_sb[:, SL * m:SL * (m + 1)], ps[:],
                                     AF.Gelu_apprx_tanh,
                                     bias=b_fc_sb[:, m:m + 1])
            for t in range(ND):
                ps = fps.tile([128, SL], F32, tag="o", bufs=2)
                for m in range(NFF):
                    nc.tensor.matmul(ps[:], w_pj_sb[m][:, 128 * t:128 * (t + 1)],
                                     h_sb[:, SL * m:SL * (m + 1)],
                                     start=(m == 0), stop=(m == NFF - 1))
                o = fp.tile([128, SL], F32, tag="out", bufs=2, name=f"o{t}")
                nc.vector.scalar_tensor_tensor(o[:], ps[:],
                                               b_proj_sb[:, t:t + 1],
                                               x2ln[t][:],
                                               op0=ALU.add, op1=ALU.add)
                nc.sync.dma_start(outT[128 * t:128 * (t + 1), :], o[:])
            fps.release()
        wffn.release()

    nc.compile()
    return nc


# ---- host side --------------------------------------------------------------

def _prep_inputs(cfg, x, ln1_w, ln1_b, w_attn, b_attn, ln2_w, ln2_b,
                 w_fc, b_fc, w_proj, b_proj):
    D, H, NC, SL, ND, NDP, NFF = (cfg.D, cfg.H, cfg.NC, cfg.SL, cfg.ND,
                                  cfg.NDP, cfg.NFF)
    import ml_dtypes
    bf16 = ml_dtypes.bfloat16
    fp8 = ml_dtypes.float8_e4m3

    def pp(v, n):
        return np.ascontiguousarray(v.reshape(n, 128).T.astype(np.float32))

    # natural column order; x4 (TRN fp8e4 max 240; with x8 activations the
    # projections come out at 32x their true values, absmax ~130)
    wsel = w_attn.T * 4.0                                  # [768, 2304]
    # DoubleRow pair layout [NDP, 128, 2, 2304]: row pairs (dp, dp+3)
    wp8 = np.empty((NDP, 128, 2, 2304), np.float32)
    for dp in range(NDP):
        for j in range(2):
            t = dp + 3 * j
            wp8[dp, :, j, :] = wsel[128 * t:128 * (t + 1), :]
    w_qkv8 = np.ascontiguousarray(
        wp8.reshape(NDP * 128, 2 * 2304).astype(fp8))

    b_q32 = pp(b_attn[0:D] * 32.0, ND)
    b_k32 = pp(b_attn[D:2 * D] * 32.0, ND)
    b_v = np.ascontiguousarray(b_attn[2 * D:].reshape(1, D).astype(np.float32))

    common = {
        "w_qkv8": w_qkv8,
        "b_q32": b_q32, "b_k32": b_k32, "b_v": b_v,
        "ln1w": pp(ln1_w, ND), "ln1b": pp(ln1_b, ND),
        "ln2w": pp(ln2_w, ND), "ln2b": pp(ln2_b, ND),
        "w_fcT": np.ascontiguousarray(w_fc.T.astype(bf16)),
        "b_fc": pp(b_fc, NFF),
        "w_projT": np.ascontiguousarray(w_proj.T.astype(bf16)),
        "b_proj": pp(b_proj, ND),
    }
    xT = np.ascontiguousarray(x.T.astype(np.float32))
    in_maps = []
    for c in range(NC):
        m = dict(common)
        m["xT"] = np.ascontiguousarray(xT[:, c * SL:(c + 1) * SL])
        in_maps.append(m)
    return in_maps


_CACHE = {}


def kernel(**inputs):
    cfg = Cfg()
    inputs = {k: np.asarray(v) for k, v in inputs.items()}
    in_maps = _prep_inputs(cfg, **inputs)
    if "nc" not in _CACHE:
        _CACHE["nc"] = build(cfg)
    nc = _CACHE["nc"]
    from concourse.bass_utils import run_bass_kernel_spmd
    res = run_bass_kernel_spmd(nc, in_maps, list(range(cfg.NC)))
    outs = [np.asarray(res.results[c]["outT"], dtype=np.float32).T
            for c in range(cfg.NC)]
    return np.ascontiguousarray(np.concatenate(outs, axis=0))


# revision 22
# speedup vs baseline: 1.4228x; 1.0763x over previous
"""Trainium2 Bass kernel for nn_DecoderBlock (S=4096, D=768, H=12).

Strategy (8 NeuronCores, SPMD, sequence-parallel: core c owns rows
[c*512, (c+1)*512)):

  - All activations transposed (features on partitions, sequence free).
    LayerNorm stats via ones-vector matmuls on bf16 copies (1 cyc/row).
  - Attention runs in fp8(e4m3, TRN flavor: max 240):
      * scores: per head-pair tile (head A dims on partitions 0:64, head
        B on 64:128), one K=64 fp8 matmul per (head, key-tile) with
        tile positions ping-ponging (0,0)/(64,0) -- measured 108ns per
        [128,512] matmul (2x over a fixed position).
      * PV: DoubleRow pairs two key-tiles per matmul (effective K=256 in
        one 216ns pass); the stationary V blocks are padded to 80 cols
        with an all-ones column at 64, so the softmax denominator
        accumulates on PSUM partition 64 for free.
  - Softmax exp splits across TWO engines: the Scalar engine computes
    lam*exp(y) (bias=ln lam) for even key-tiles while the Vector engine
    computes the same for odd tiles via a custom 6-stage DVE op
    (((c0*y+c1)^2+c2)^2)^2 ~= lam*exp(y) (+-2.8% shape error; the
    common lam cancels in the softmax normalization).
  - x is pre-scaled x8 and qkv weights x4 (TRN fp8 max is 240; q/k/v
    then sit at 32x true value, absmax ~130). The 1/32 folds into the
    denominator reciprocal broadcast; 1/(32*32) and softmax 1/sqrt(64)
    fold into the exp input scale.
  - K and V are cast to fp8 and exchanged with 3 chunked AllGathers
    (one per 4-head group) so attention starts while later chunks fly.
  - QKV projections use fp8 DoubleRow over d-tile pairs (half the
    matmul count); FFN stays bf16 (fp8 would blow the 2e-2 budget);
    FFN weights are prefetched during attention.
"""

import os
import sys

for _p in ("/opt/trn_rl_repo", os.path.expanduser("~/.axon_site/_ro/trn_rl_repo")):
    if os.path.isdir(_p) and _p not in sys.path:
        sys.path.append(_p)

import numpy as np
from contextlib import ExitStack

import concourse.bass as bass
import concourse.tile as tile
from concourse import bacc, mybir
import concourse.dve_ops as dve_ops
from concourse.dve_spec import Spec, Src0, C0, C1, C2, sq, lower as dve_lower
from concourse.dve_uop import DveOpSpec

F32 = mybir.dt.float32
BF16 = mybir.dt.bfloat16
FP8 = mybir.dt.float8e4
AF = mybir.ActivationFunctionType
ALU = mybir.AluOpType
DR = mybir.MatmulPerfMode.DoubleRow

# exp-poly coefficients: ((EC0*y+EC1)^2+EC2)^4 ~= ELAM * e^y on |y|<=2.2
EC0, EC1, EC2 = 0.19508098, 0.82453421, 0.56463811
ELAM = 2.3759038641914842


def _ref_exp_poly(in0, in1, s0, s1, imm2):
    t = (in0.astype(np.float32) * s0 + s1) ** 2 + imm2
    return (t * t) ** 2


def register_exp_poly():
    for op in dve_ops.OPS:
        if op.name == "EXP_POLY_ANT":
            return op
    spec = Spec(body=sq(sq(sq(Src0 * C0 + C1) + C2)), reference=_ref_exp_poly)
    name = "EXP_POLY_ANT"
    opcode = dve_ops._CUSTOM_DVE_ROW_BASE + len(dve_ops.OPS)
    shas = {}
    for ver in ("v3", "v4"):
        try:
            s = DveOpSpec(name=name, opcode=opcode,
                          uops=dve_lower(spec, ver=ver), rd1_en=False)
            shas[ver] = s.sha(ver)
        except Exception:
            pass
    op = dve_ops.DveOp(name, spec, subdim=False, uops_sha=shas)
    dve_ops.OPS.append(op)
    dve_ops.CUSTOM_DVE_SPECS[name] = op.spec
    dve_ops._SUB_OPCODE_FOR_NAME[name] = opcode
    return op


EXP_POLY = register_exp_poly()


class Cfg:
    def __init__(self, S=4096, D=768, H=12, NC=8, eps=1e-5):
        self.S, self.D, self.H, self.NC, self.eps = S, D, H, NC, eps
        self.DH = D // H              # 64
        self.SL = S // NC             # 512 local rows
        self.ND = D // 128            # 6 d-tiles
        self.NDP = self.ND // 2       # 3 d-tile pairs
        self.NSK = S // 128           # 32 key tiles
        self.NPAIR = self.NSK // 2    # 16 key-tile pairs
        self.NG = 3                   # head groups (4 heads each)
        self.NHP = H // 2             # 6 head pairs
        self.NFF = 4 * D // 128       # 24 ffn tiles
        self.scale = 1.0 / float(np.sqrt(self.DH))
        self.yscale = self.scale / 1024.0   # q8,k8 are 32x the true values


def build(cfg: Cfg, debug=False, enable_asserts=False, dbg_dump=False):
    nc = bacc.Bacc(
        "TRN2",
        target_bir_lowering=False,
        debug=debug,
        enable_asserts=enable_asserts,
        num_devices=cfg.NC,
    )
    S, D, H, SL = cfg.S, cfg.D, cfg.H, cfg.SL
    ND, NDP, NSK, NPAIR, NG, NHP, NFF, NC = (
        cfg.ND, cfg.NDP, cfg.NSK, cfg.NPAIR, cfg.NG, cfg.NHP, cfg.NFF, cfg.NC)

    # ---- DRAM I/O ----------------------------------------------------------
    xT = nc.dram_tensor("xT", [D, SL], F32, kind="ExternalInput").ap()
    w_qkv8 = nc.dram_tensor("w_qkv8", [NDP * 128, 2 * 2304], FP8,
                            kind="ExternalInput").ap()
    b_q32 = nc.dram_tensor("b_q32", [128, ND], F32, kind="ExternalInput").ap()
    b_k32 = nc.dram_tensor("b_k32", [128, ND], F32, kind="ExternalInput").ap()
    b_v = nc.dram_tensor("b_v", [1, D], F32, kind="ExternalInput").ap()
    ln1w = nc.dram_tensor("ln1w", [128, ND], F32, kind="ExternalInput").ap()
    ln1b = nc.dram_tensor("ln1b", [128, ND], F32, kind="ExternalInput").ap()
    ln2w = nc.dram_tensor("ln2w", [128, ND], F32, kind="ExternalInput").ap()
    ln2b = nc.dram_tensor("ln2b", [128, ND], F32, kind="ExternalInput").ap()
    w_fcT = nc.dram_tensor("w_fcT", [D, 4 * D], BF16, kind="ExternalInput").ap()
    b_fc = nc.dram_tensor("b_fc", [128, NFF], F32, kind="ExternalInput").ap()
    w_projT = nc.dram_tensor("w_projT", [4 * D, D], BF16,
                             kind="ExternalInput").ap()
    b_proj = nc.dram_tensor("b_proj", [128, ND], F32, kind="ExternalInput").ap()
    outT = nc.dram_tensor("outT", [D, SL], F32, kind="ExternalOutput").ap()
    if dbg_dump:
        d_q8 = nc.dram_tensor("d_q8", [128, SL], FP8, kind="ExternalOutput").ap()
        d_kT = nc.dram_tensor("d_kT", [128, S], FP8, kind="ExternalOutput").ap()
        d_vall = nc.dram_tensor("d_vall", [128, H * (S // 256) * 160], FP8,
                                kind="ExternalOutput").ap()  # only first 1/3 used
        d_ex = nc.dram_tensor("d_ex", [128, 4 * SL], FP8, kind="ExternalOutput").ap()
        d_ctxS = nc.dram_tensor("d_ctxS", [64, SL], F32, kind="ExternalOutput").ap()
        d_den = nc.dram_tensor("d_den", [1, SL], F32, kind="ExternalOutput").ap()
        d_x2 = nc.dram_tensor("d_x2", [128, SL], F32, kind="ExternalOutput").ap()

    with tile.TileContext(nc) as tc, ExitStack() as top:
        persist = top.enter_context(tc.tile_pool(name="persist", bufs=1))
        dram = top.enter_context(tc.tile_pool(name="dram", bufs=1, space="DRAM"))

        ones_colb = persist.tile([128, 1], BF16)
        nc.vector.memset(ones_colb[:], 1.0)
        ones_row32 = persist.tile([1, 128], F32)
        nc.vector.memset(ones_row32[:], 32.0)
        ones128b = persist.tile([1, 128], BF16)
        nc.vector.memset(ones128b[:], 1.0)
        eps_tile = persist.tile([1, 1], F32)
        nc.vector.memset(eps_tile[:], float(cfg.eps))
        lnlam = persist.tile([128, 1], F32)
        nc.vector.memset(lnlam[:], float(np.log(ELAM)))
        ones64f = persist.tile([1, 64], BF16)
        nc.vector.memset(ones64f[:], 1.0 / 32.0)

        b_q32_sb = persist.tile([128, ND], F32)
        nc.sync.dma_start(b_q32_sb[:], b_q32[:])
        b_k32_sb = persist.tile([128, ND], F32)
        nc.sync.dma_start(b_k32_sb[:], b_k32[:])
        b_v_sb = persist.tile([1, D], F32)
        nc.sync.dma_start(b_v_sb[:], b_v[:])
        ln1w_sb = persist.tile([128, ND], F32)
        nc.sync.dma_start(ln1w_sb[:], ln1w[:])
        ln1b_sb = persist.tile([128, ND], F32)
        nc.sync.dma_start(ln1b_sb[:], ln1b[:])
        ln2w_sb = persist.tile([128, ND], F32)
        nc.sync.dma_start(ln2w_sb[:], ln2w[:])
        ln2b_sb = persist.tile([128, ND], F32)
        nc.sync.dma_start(ln2b_sb[:], ln2b[:])
        b_fc_sb = persist.tile([128, NFF], F32)
        nc.sync.dma_start(b_fc_sb[:], b_fc[:])
        b_proj_sb = persist.tile([128, ND], F32)
        nc.sync.dma_start(b_proj_sb[:], b_proj[:])

        ln1w8 = persist.tile([128, ND], F32)
        nc.vector.tensor_scalar_mul(ln1w8[:], ln1w_sb[:], 8.0)
        ln1b8 = persist.tile([128, ND], F32)
        nc.vector.tensor_scalar_mul(ln1b8[:], ln1b_sb[:], 8.0)

        # persistent activations
        ln1x = [persist.tile([128, SL], F32, name=f"ln1x{t}") for t in range(ND)]
        x2bf = [persist.tile([128, SL], BF16, name=f"x2bf{t}")
                for t in range(ND)]
        x2 = [persist.tile([128, SL], F32, name=f"x2_{t}") for t in range(ND)]
        q8 = [persist.tile([128, SL], FP8, name=f"q8_{hp}") for hp in range(NHP)]

        def layernorm_T(src_f32, src_bf, w_sb, b_sb, wq_sb, bq_sb, q_dtype,
                        out_f32, out_q):
            """LN over the partition (feature) axis; stats on bf16 copies."""
            with tc.tile_pool(name="ln_ps", bufs=1, space="PSUM") as lps, \
                 tc.tile_pool(name="ln_sb", bufs=2) as lsb:
                sums = lps.tile([1, SL], F32, tag="st", bufs=2)
                sumsq = lps.tile([1, SL], F32, tag="st", bufs=2)
                sq_ = [lsb.tile([128, SL], BF16, tag="lnsq", bufs=2,
                                name=f"sq{t}") for t in range(ND)]
                for t in range(ND):
                    nc.gpsimd.tensor_tensor(sq_[t][:], src_bf[t][:],
                                            src_bf[t][:], op=ALU.mult)
                for t in range(ND):
                    nc.tensor.matmul(sums[:], ones_colb[:], src_bf[t][:],
                                     start=(t == 0), stop=(t == ND - 1))
                for t in range(ND):
                    nc.tensor.matmul(sumsq[:], ones_colb[:], sq_[t][:],
                                     start=(t == 0), stop=(t == ND - 1))
                mean = lsb.tile([1, SL], F32)
                ex2 = lsb.tile([1, SL], F32)
                msq = lsb.tile([1, SL], F32)
                var = lsb.tile([1, SL], F32)
                lnv = lsb.tile([1, SL], F32)
                meanb = lsb.tile([1, SL], BF16)
                rstdb = lsb.tile([1, SL], BF16)
                nc.vector.tensor_scalar_mul(mean[:], sums[:], 1.0 / D)
                nc.vector.tensor_scalar_mul(ex2[:], sumsq[:], 1.0 / D)
                nc.vector.tensor_tensor(msq[:], mean[:], mean[:], op=ALU.mult)
                nc.vector.tensor_tensor(var[:], ex2[:], msq[:], op=ALU.subtract)
                nc.scalar.activation(lnv[:], var[:], AF.Ln, bias=eps_tile[:])
                nc.scalar.activation(rstdb[:], lnv[:], AF.Exp, scale=-0.5)
                nc.vector.tensor_copy(meanb[:], mean[:])
                with tc.tile_pool(name="lnb_ps", bufs=1, space="PSUM") as bps:
                    meanB = bps.tile([128, SL], F32, tag="bc", bufs=2)
                    rstdB = bps.tile([128, SL], F32, tag="bc", bufs=2)
                    nc.tensor.matmul(meanB[:], ones128b[:], meanb[:],
                                     start=True, stop=True)
                    nc.tensor.matmul(rstdB[:], ones128b[:], rstdb[:],
                                     start=True, stop=True)
                    # gpsimd can't read PSUM: stage broadcasts in SBUF
                    meanS = lsb.tile([128, SL], F32)
                    rstdS = lsb.tile([128, SL], F32)
                    nc.vector.tensor_copy(meanS[:], meanB[:])
                    nc.vector.tensor_copy(rstdS[:], rstdB[:])
                    for t in range(ND):
                        cen = lsb.tile([128, SL], F32, tag="lncen", bufs=2,
                                       name=f"cen{t}")
                        nc.gpsimd.tensor_tensor(cen[:], src_f32[t][:],
                                                meanS[:], op=ALU.subtract)
                        nc.gpsimd.tensor_tensor(cen[:], cen[:], rstdS[:],
                                                op=ALU.mult)
                        nc.scalar.activation(out_f32[t][:], cen[:], AF.Identity,
                                             bias=b_sb[:, t:t + 1],
                                             scale=w_sb[:, t:t + 1])
                        nc.gpsimd.tensor_scalar(out_q[:, SL * t:SL * (t + 1)],
                                                cen[:], wq_sb[:, t:t + 1],
                                                bq_sb[:, t:t + 1],
                                                op0=ALU.mult, op1=ALU.add)

        # ==== P0: LN1 =======================================================
        ln1x8 = persist.tile([128, ND * SL], FP8)
        with tc.tile_pool(name="xin", bufs=1) as xin:
            x_sb = [xin.tile([128, SL], F32, name=f"x_sb{t}") for t in range(ND)]
            x_bf = [xin.tile([128, SL], BF16, name=f"x_bf{t}") for t in range(ND)]
            for t in range(ND):
                nc.sync.dma_start(x_sb[t][:], xT[128 * t:128 * (t + 1), :])
                nc.gpsimd.dma_start(x_bf[t][:], xT[128 * t:128 * (t + 1), :])
            layernorm_T(x_sb, x_bf, ln1w_sb, ln1b_sb, ln1w8, ln1b8, FP8,
                        ln1x, ln1x8)

        # ==== P1: k,v (chunked by head group) + allgather; then q ===========
        # kv_own[g]: [128, 2048]: cols 0:1024 k (2 head-pairs), 1024: v
        kv_own = [dram.tile([128 * 2048], FP8, name=f"kv_own{g}")
                  for g in range(NG)]
        gspace = "Shared" if NC > 4 else "Local"
        kv_gath = [dram.tile([NC * 128 * 2048], FP8, addr_space=gspace,
                             name=f"kv_gath{g}") for g in range(NG)]
        grp = [list(range(NC))]

        def ln1x8_pair(dp, lo, n):
            """AP [128, 2, n] pairing d-tiles (dp, dp+3), cols lo:lo+n."""
            return ln1x8.rearrange("p (two c) -> p two c", two=2)[
                :, :, SL * dp + lo:SL * dp + lo + n]

        with tc.tile_pool(name="wqkv", bufs=1) as wp, \
             tc.tile_pool(name="qkv_ps", bufs=1, space="PSUM") as qps, \
             tc.tile_pool(name="kv_sb", bufs=1) as kvp:
            w8 = [wp.tile([128, 2 * 2304], FP8, name=f"w8_{dp}")
                  for dp in range(NDP)]
            for dp in range(NDP):
                nc.sync.dma_start(w8[dp][:],
                                  w_qkv8[128 * dp:128 * (dp + 1), :])

            def wq_ap(dp, base, n):
                return w8[dp].rearrange("p (two c) -> p two c", two=2)[
                    :, :, base:base + n]

            # bvb32: v-bias broadcast to [128, D], x32
            bvb32 = kvp.tile([128, D], F32)
            with tc.tile_pool(name="bv_ps", bufs=1, space="PSUM") as bvp:
                for i in range(2):
                    bvb = bvp.tile([128, 512], F32, tag="bvb", bufs=2,
                                   name=f"bvb{i}")
                    nc.tensor.matmul(bvb[:, 0:384], ones_row32[:],
                                     b_v_sb[:, 384 * i:384 * (i + 1)],
                                     start=True, stop=True)
                    nc.vector.tensor_copy(bvb32[:, 384 * i:384 * (i + 1)],
                                          bvb[:, 0:384])

            for g in range(NG):
                kown = kv_own[g].rearrange("(p c) -> p c", p=128)
                for i in range(2):         # head pair 2g+i
                    hp = 2 * g + i
                    ps = qps.tile([128, SL], F32, tag="qk", bufs=3)
                    for dp in range(NDP):
                        nc.tensor.matmul(
                            ps[:], wq_ap(dp, 768 + 128 * hp, 128),
                            ln1x8_pair(dp, 0, SL), start=(dp == 0),
                            stop=(dp == NDP - 1), perf_mode=DR)
                    k8t = kvp.tile([128, SL], FP8, tag="k8", bufs=2,
                                   name=f"k8_{g}_{i}")
                    nc.scalar.activation(
                        k8t[:], ps[:], AF.Identity,
                        bias=b_k32_sb[:, hp:hp + 1])
                    nc.sync.dma_start(kown[:, 512 * i:512 * (i + 1)], k8t[:])
                for m in range(4):
                    ps = qps.tile([128, 512], F32, tag="v", bufs=3)
                    pv = ps[:, 0:256]
                    for dp in range(NDP):
                        nc.tensor.matmul(
                            pv, ln1x8_pair(dp, 128 * m, 128),
                            wq_ap(dp, 1536 + 256 * g, 256),
                            start=(dp == 0), stop=(dp == NDP - 1), perf_mode=DR)
                    v8t = kvp.tile([128, 256], FP8, tag="v8", bufs=3,
                                   name=f"v8_{g}_{m}")
                    nc.vector.scalar_tensor_tensor(
                        v8t[:], pv, 1.0, bvb32[:, 256 * g:256 * (g + 1)],
                        op0=ALU.mult, op1=ALU.add)
                    nc.sync.dma_start(
                        kown[:, 1024 + 256 * m:1024 + 256 * (m + 1)], v8t[:])
                nc.gpsimd.collective_compute(
                    "AllGather", ALU.bypass, replica_groups=grp,
                    ins=[kv_own[g][:]], outs=[kv_gath[g][:]])

            for hp in range(NHP):
                ps = qps.tile([128, SL], F32, tag="qk", bufs=3)
                for dp in range(NDP):
                    nc.tensor.matmul(
                        ps[:], wq_ap(dp, 128 * hp, 128),
                        ln1x8_pair(dp, 0, SL), start=(dp == 0),
                        stop=(dp == NDP - 1), perf_mode=DR)
                nc.scalar.activation(
                    q8[hp][:], ps[:], AF.Identity,
                    bias=b_q32_sb[:, hp:hp + 1])

        # ==== P2+P3: attention ==============================================
        wffn = tc.alloc_tile_pool(name="wffn", bufs=1)
        w_fc_sb = [wffn.tile([128, 4 * D], BF16, name=f"wfc{t}")
                   for t in range(ND)]
        w_pj_sb = [wffn.tile([128, D], BF16, name=f"wpj{m}") for m in range(NFF)]
        attn = tc.alloc_tile_pool(name="attn", bufs=1)

        kT = [attn.tile([128, S], FP8, name=f"kT{hp}") for hp in range(NHP)]
        # v_g[g]: 4 heads; per head h'(0..3), key-pair p, i: 80-col block
        # col = h'*2560 + p*160 + i*80 + d ; col d=64 is all-ones
        v_g = [attn.tile([128, 4 * NPAIR * 160], FP8, name=f"v_g{g}")
               for g in range(NG)]
        for g in range(NG):
            if dbg_dump:
                nc.vector.memset(v_g[g][:], 0.0)
            nc.vector.memset(
                v_g[g].rearrange("p (blk w) -> p blk w", w=80)[:, :, 64:65],
                1.0)
            gath = kv_gath[g].rearrange("(c p w) -> c p w", c=NC, p=128)
            for i in range(2):
                hp = 2 * g + i
                kdst = kT[hp].rearrange("p (c s) -> p c s", c=NC)
                for c in range(NC):
                    nc.sync.dma_start(kdst[:, c, :],
                                      gath[c, :, 512 * i:512 * (i + 1)])
            for c in range(NC):
                for m in range(4):
                    b = 4 * c + m
                    p_, i_ = b // 2, b % 2
                    vdst = v_g[g].rearrange(
                        "p (h pp w) -> p h pp w", h=4, pp=NPAIR)[
                        :, :, p_, 80 * i_:80 * i_ + 64]
                    vsrc = gath[c, :, 1024 + 256 * m:1024 + 256 * (m + 1)
                                ].rearrange("p (h d) -> p h d", h=4)
                    nc.sync.dma_start(vdst, vsrc)

        if dbg_dump:
            nc.sync.dma_start(d_q8[:], q8[0][:])
            nc.sync.dma_start(d_kT[:], kT[0][:])
            nc.sync.dma_start(d_vall[:, 0:4 * (S // 256) * 160], v_g[0][:])
        # prefetch FFN weights now: issued after the kv unpack DMAs so the
        # gather/unpack wins the queue race; still hidden under attention
        for t in range(ND):
            nc.sync.dma_start(w_fc_sb[t][:], w_fcT[128 * t:128 * (t + 1), :])
        for m in range(NFF):
            nc.sync.dma_start(w_pj_sb[m][:], w_projT[128 * m:128 * (m + 1), :])

        with tc.tile_pool(name="sg_ps", bufs=1, space="PSUM") as sps, \
             tc.tile_pool(name="cs_ps", bufs=1, space="PSUM") as cps, \
             tc.tile_pool(name="exp_sb", bufs=1) as epool:
            for hp in range(NHP):
                ha, hb = 2 * hp, 2 * hp + 1
                vv = v_g[hp // 2].rearrange("p (hq two w) -> p hq two w",
                                            two=2, w=80)
                ctxA = cps.tile([65, SL], F32, tag="ctxA", bufs=1)
                ctxB = cps.tile([65, SL], F32, tag="ctxB", bufs=1)
                for p_ in range(NPAIR):
                    ex = epool.tile([128, 2 * 2 * SL], FP8, tag="ex", bufs=3)
                    for i in range(2):
                        b = 2 * p_ + i
                        sg = sps.tile([128, 2 * SL], F32, tag="sg", bufs=3)
                        nc.tensor.matmul(
                            sg[:, 0:SL],
                            kT[hp][0:64, 128 * b:128 * (b + 1)],
                            q8[hp][0:64, :], start=True, stop=True,
                            tile_position=(0, 0))
                        nc.tensor.matmul(
                            sg[:, SL:2 * SL],
                            kT[hp][64:128, 128 * b:128 * (b + 1)],
                            q8[hp][64:128, :], start=True, stop=True,
                            tile_position=(64, 0))
                        if (hp * NSK + b) % 2 == 0:
                            nc.scalar.activation(
                                ex[:, 1024 * i:1024 * (i + 1)], sg[:], AF.Exp,
                                scale=cfg.yscale, bias=lnlam[:])
                        else:
                            nc.vector._custom_dve(
                                EXP_POLY, out=ex[:, 1024 * i:1024 * (i + 1)],
                                in0=sg[:], s0=EC0 * cfg.yscale,
                                s1=EC1, imm2=EC2)
                    if dbg_dump and hp == 0 and p_ == 0:
                        nc.sync.dma_start(d_ex[:], ex[:])
                    # ex layout: [i(2), head(2), s(512)]
                    exr = ex.rearrange("p (two hq s) -> p two hq s",
                                       two=2, hq=2)
                    nc.tensor.matmul(ctxA[:],
                                     vv[:, (ha % 4) * NPAIR + p_, :, 0:65],
                                     exr[:, :, 0, :],
                                     start=(p_ == 0), stop=(p_ == NPAIR - 1),
                                     perf_mode=DR)
                    nc.tensor.matmul(ctxB[:],
                                     vv[:, (hb % 4) * NPAIR + p_, :, 0:65],
                                     exr[:, :, 1, :],
                                     start=(p_ == 0), stop=(p_ == NPAIR - 1),
                                     perf_mode=DR)
                # epilogue: r = 1/(32*den); x2 = ln1x + ctx*r; head B lands on
                # partitions 64:128 via a partition-shift DMA.
                for (half, ctx) in ((0, ctxA), (1, ctxB)):
                    ctxS = epool.tile([64, SL], F32,
                                      tag="ctxS", bufs=2)
                    nc.vector.tensor_copy(ctxS[:], ctx[0:64, :])
                    den_s = epool.tile([1, SL], F32,
                                       tag="den_s", bufs=2)
                    nc.vector.tensor_copy(den_s[:], ctx[64:65, :])
                    den_r = epool.tile([1, SL], F32,
                                       tag="den_r", bufs=2)
                    nc.vector.reciprocal_approx_fast(den_r[:], den_s[:])
                    den_rb = epool.tile([1, SL], BF16,
                                        tag="den_rb", bufs=2)
                    nc.gpsimd.tensor_copy(den_rb[:], den_r[:])
                    rb = sps.tile([128, 2 * SL], F32, tag="sg", bufs=3)
                    nc.tensor.matmul(rb[0:64, 0:SL], ones64f[:], den_rb[:],
                                     start=True, stop=True)
                    if dbg_dump and hp == 0 and half == 0:
                        nc.sync.dma_start(d_ctxS[:], ctxS[:])
                        nc.sync.dma_start(d_den[:], den_r[:])
                    cn = epool.tile([64, SL], F32, tag="cn", bufs=2)
                    nc.vector.scalar_tensor_tensor(cn[:], rb[0:64, 0:SL], 1.0,
                                                   ctxS[:],
                                                   op0=ALU.mult, op1=ALU.mult)
                    if half == 0:
                        nc.gpsimd.tensor_tensor(x2[hp][0:64, :], cn[:],
                                                ln1x[hp][0:64, :], op=ALU.add)
                        nc.gpsimd.tensor_copy(x2bf[hp][0:64, :],
                                              x2[hp][0:64, :])
                    else:
                        nc.sync.dma_start(x2[hp][64:128, :], cn[:])
                        nc.gpsimd.tensor_tensor(x2[hp][64:128, :],
                                                x2[hp][64:128, :],
                                                ln1x[hp][64:128, :],
                                                op=ALU.add)
                        nc.gpsimd.tensor_copy(x2bf[hp][64:128, :],
                                              x2[hp][64:128, :])
        if dbg_dump:
            nc.sync.dma_start(d_x2[:], x2[0][:])
        attn.release()

        # ==== P4+P5: LN2 + FFN =============================================
        with tc.tile_pool(name="ffn_sb", bufs=1) as fp:
            x2ln = ln1x   # reuse
            x2lnb = fp.tile([128, ND * SL], BF16)
            layernorm_T(x2, x2bf, ln2w_sb, ln2b_sb, ln2w_sb, ln2b_sb, BF16,
                        x2ln, x2lnb)
            fps = tc.alloc_tile_pool(name="ffn_ps", bufs=1, space="PSUM")
            h_sb = fp.tile([128, NFF * SL], BF16)
            for m in range(NFF):
                ps = fps.tile([128, SL], F32, tag="h", bufs=4)
                for t in range(ND):
                    nc.tensor.matmul(ps[:], w_fc_sb[t][:, 128 * m:128 * (m + 1)],
                                     x2lnb[:, SL * t:SL * (t + 1)],
                                     start=(t == 0), stop=(t == ND - 1))
                nc.scalar.activation(h_sb[:, SL * m:SL * (m + 1)], ps[:],
                                     AF.Gelu_apprx_tanh,
                                     bias=b_fc_sb[:, m:m + 1])
            for t in range(ND):
                ps = fps.tile([128, SL], F32, tag="o", bufs=2)
                for m in range(NFF):
                    nc.tensor.matmul(ps[:], w_pj_sb[m][:, 128 * t:128 * (t + 1)],
                                     h_sb[:, SL * m:SL * (m + 1)],
                                     start=(m == 0), stop=(m == NFF - 1))
                o = fp.tile([128, SL], F32, tag="out", bufs=2, name=f"o{t}")
                nc.vector.scalar_tensor_tensor(o[:], ps[:],
                                               b_proj_sb[:, t:t + 1],
                                               x2ln[t][:],
                                               op0=ALU.add, op1=ALU.add)
                nc.sync.dma_start(outT[128 * t:128 * (t + 1), :], o[:])
            fps.release()
        wffn.release()

    nc.compile()
    return nc


# ---- host side --------------------------------------------------------------

def _prep_inputs(cfg, x, ln1_w, ln1_b, w_attn, b_attn, ln2_w, ln2_b,
                 w_fc, b_fc, w_proj, b_proj):
    D, H, NC, SL, ND, NDP, NFF = (cfg.D, cfg.H, cfg.NC, cfg.SL, cfg.ND,
                                  cfg.NDP, cfg.NFF)
    import ml_dtypes
    bf16 = ml_dtypes.bfloat16
    fp8 = ml_dtypes.float8_e4m3

    def pp(v, n):
        return np.ascontiguousarray(v.reshape(n, 128).T.astype(np.float32))

    # natural column order; x4 (TRN fp8e4 max 240; with x8 activations the
    # projections come out at 32x their true values, absmax ~130)
    wsel = w_attn.T * 4.0                                  # [768, 2304]
    # DoubleRow pair layout [NDP, 128, 2, 2304]: row pairs (dp, dp+3)
    wp8 = np.empty((NDP, 128, 2, 2304), np.float32)
    for dp in range(NDP):
        for j in range(2):
            t = dp + 3 * j
            wp8[dp, :, j, :] = wsel[128 * t:128 * (t + 1), :]
    w_qkv8 = np.ascontiguousarray(
        wp8.reshape(NDP * 128, 2 * 2304).astype(fp8))

    b_q32 = pp(b_attn[0:D] * 32.0, ND)
    b_k32 = pp(b_attn[D:2 * D] * 32.0, ND)
    b_v = np.ascontiguousarray(b_attn[2 * D:].reshape(1, D).astype(np.float32))

    common = {
        "w_qkv8": w_qkv8,
        "b_q32": b_q32, "b_k32": b_k32, "b_v": b_v,
        "ln1w": pp(ln1_w, ND), "ln1b": pp(ln1_b, ND),
        "ln2w": pp(ln2_w, ND), "ln2b": pp(ln2_b, ND),
        "w_fcT": np.ascontiguousarray(w_fc.T.astype(bf16)),
        "b_fc": pp(b_fc, NFF),
        "w_projT": np.ascontiguousarray(w_proj.T.astype(bf16)),
        "b_proj": pp(b_proj, ND),
    }
    xT = np.ascontiguousarray(x.T.astype(np.float32))
    in_maps = []
    for c in range(NC):
        m = dict(common)
        m["xT"] = np.ascontiguousarray(xT[:, c * SL:(c + 1) * SL])
        in_maps.append(m)
    return in_maps


_CACHE = {}


def kernel(**inputs):
    cfg = Cfg()
    inputs = {k: np.asarray(v) for k, v in inputs.items()}
    in_maps = _prep_inputs(cfg, **inputs)
    if "nc" not in _CACHE:
        _CACHE["nc"] = build(cfg)
    nc = _CACHE["nc"]
    from concourse.bass_utils import run_bass_kernel_spmd
    res = run_bass_kernel_spmd(nc, in_maps, list(range(cfg.NC)))
    outs = [np.asarray(res.results[c]["outT"], dtype=np.float32).T
            for c in range(cfg.NC)]
    return np.ascontiguousarray(np.concatenate(outs, axis=0))
